# revision 40
# baseline (speedup 1.0000x reference)
"""Trainium2 Bass kernel for a 2-layer GCN + linear classifier (PyG GCNConv style).

Self-contained: hardcodes the 8-core sharding strategy; all graph/index
preprocessing is host-side numpy, all FLOPs on x run on device.

Sharding: nodes are split into 8 contiguous canonical blocks (one per core,
padded to a multiple of 128; slot 0 of each block is a guaranteed-zero pad
row). Per GCN layer each core computes its block's dense transform (bf16
PE matmuls, f32 PSUM), pre-scales rows by dinv, and two AllGathers (kicked
per table half, interleaved with the transform) materialize the full bf16
node table in every core's HBM. Each core aggregates its own destinations'
in-edges with gpsimd dma_gather (256B bf16 rows, <=1024 indices per
instruction — ucode descriptor-ring limit — over 4 SWDGE queues) plus
DVE/ACT tree reductions, in two passes split by physical table half so
gather indices fit int16. Destinations are degree-sorted per (core, pass)
to minimize slab padding; a small canonicalization gather restores node
order via a bf16 partial-sum round trip through DRAM.

Pool-engine descriptor generation (~6.5ns/idx per queue, 4 queues max) is
the bottleneck. To hide the serial transform/AllGather windows, the first
gather calls of each layer's pass A — and the canonicalization gathers —
are issued as PREPARE_ONLY preps (desc-gen runs before the table data
exists; deps defer to trigger_dma). Their DMA completion is gated by
per-call user semaphores + explicit consumer wait_ge (Tile's automatic
DMASW tracking pre-bumps lane sems at prep time, so it cannot order
consumers of prepped gathers).
"""

import os
import sys
import types

import numpy as np


def _setup_env():
    if "/opt/trn_rl_repo" not in sys.path:
        sys.path.insert(0, "/opt/trn_rl_repo")
    if "antenv.axon_hooks" not in sys.modules:
        try:
            from trn_agent_boot.trn_boot import _ntff_profile_via_ctypes

            _hook = _ntff_profile_via_ctypes("/opt/axon/libaxon_pjrt.so")
        except Exception:
            _hook = None
        _mod = types.ModuleType("antenv.axon_hooks")
        _mod.get_axon_ntff_profile_hook = lambda: _hook
        _mod.set_axon_ntff_profile_hook = lambda h: None
        sys.modules["antenv.axon_hooks"] = _mod


_setup_env()

import ml_dtypes  # noqa: E402
from concourse import bacc, bass, mybir, tile  # noqa: E402
import concourse.bass_utils as bass_utils  # noqa: E402
from concourse.bass_utils import run_bass_kernel_spmd  # noqa: E402
from concourse.masks import make_identity  # noqa: E402

bass_utils.upload_artifacts = lambda tmpdir: tmpdir

# --- queue-aware DMASW semaphore lane assignment -----------------------------
# Tile assigns Pool-engine DMA instructions to the 8 DMASW semaphore lanes
# round-robin in *scheduled* order, but each lane gets locked to the SWDGE
# queue of the first instruction using it. With multi-queue dma_gather this
# races; pin each queue to its own lane subset instead.
import concourse.tile_sem_assignment as _tsa  # noqa: E402
from concourse.bass_isa import UserSyncedRemoteDMADescs as _URD  # noqa: E402
from concourse.tile_sem_assignment import DMAInst as _DMAInst  # noqa: E402

_orig_assign_tick = _tsa.TileClockTick._assign_tick


def _queue_aware_assign_tick(self, inst):
    if (
        isinstance(inst, _DMAInst)
        and not isinstance(inst, _URD)
        and inst.engine == mybir.EngineType.Pool
    ):
        q = getattr(inst, "queue_num", 0) or 0
        lanes = max(1, self.swdge_sem_count // NQ)
        rot = self.__dict__.setdefault("_q_lane_rot", {})
        r = rot.get(q, 0)
        self.next_sw_dma_idx = (q * lanes + r) % self.swdge_sem_count
        rot[q] = (r + 1) % lanes
    return _orig_assign_tick(self, inst)


_tsa.TileClockTick._assign_tick = _queue_aware_assign_tick
# -----------------------------------------------------------------------------

N_CORES = 8
P = 128
CHUNK = 8   # max gather slabs (of 128 rows) per dma_gather instruction
            # (hard ucode limit: 1024 idxs per instruction)
NQ = int(os.environ.get("KNQ", "4"))  # SWDGE queues (desc-gen parallelism)
GBUFS = int(os.environ.get("KGBUFS", "7"))
SCAP = int(os.environ.get("KSCAP", "24"))  # max staging slabs per group
PREPN = int(os.environ.get("KPREPN", "16"))  # prepare_only calls per boundary
TBL16 = os.environ.get("KTBL16", "1") == "1"  # bf16 gather table
SHADOW = os.environ.get("KSHADOW", "1") == "1"  # alias tables for early desc-gen

dt = mybir.dt
BF16 = ml_dtypes.bfloat16


# ----------------------------------------------------------------------------
# Host-side preprocessing
# ----------------------------------------------------------------------------

def _wrap16(flat: np.ndarray) -> np.ndarray:
    """Lay out an index list in dma_gather's [128, n/16] wrapped format."""
    n = flat.shape[0]
    assert n % 16 == 0
    w = flat.reshape(n // 16, 16).T.astype(np.int16)  # [16, n//16]
    return np.tile(w, (8, 1))  # replicate across the 8 groups of 16 partitions


def _preprocess(x, edge_index, W1, b1, W2, b2, Wfc, bfc):
    N, IN = x.shape
    HID = W1.shape[1]
    CLS = Wfc.shape[1]
    E = edge_index.shape[1]
    assert IN % P == 0 and HID == P

    BLK_RAW = -(-N // N_CORES)            # nodes per core before padding
    BLK = -(-BLK_RAW // P) * P            # padded block size
    assert BLK_RAW + 1 <= BLK, "need pad slots per block"
    NPAD = N_CORES * BLK
    MT = BLK // P
    MTA = MT // 2                         # tiles per block in table half A
    HA = MTA * P                          # rows per block in half A
    HB = BLK - HA
    NROWSA = N_CORES * HA                 # physical half-A table rows
    NROWSB = N_CORES * HB
    assert NROWSA < 32768 and NROWSB < 32768

    src = edge_index[0].astype(np.int64)
    dst = edge_index[1].astype(np.int64)

    deg = np.bincount(dst, minlength=N).astype(np.float64) + 1.0
    dinv = (1.0 / np.sqrt(deg)).astype(np.float32)
    dinv_c = np.zeros(NPAD, dtype=np.float32)
    all_ids = np.arange(N, dtype=np.int64)
    # block-local slot: j=0 reserved as a guaranteed-zero pad row (half A),
    # reals at j in [1, BLK_RAW], remaining pads at the tail (half B).
    canon = (all_ids // BLK_RAW) * BLK + 1 + (all_ids % BLK_RAW)
    dinv_c[canon] = dinv

    def phys(c):
        r = c // BLK
        j = c % BLK
        return np.where(j < HA, r * HA + j, NROWSA + r * HB + (j - HA))

    ZROW_A = 0                              # block 0, j=0
    assert BLK_RAW + 1 < BLK, "need a tail pad slot per block"
    assert BLK_RAW + 1 >= HA, "tail pads must land in half B"
    ZROW_B = int(phys(np.array([BLK_RAW + 1]))[0] - NROWSA)

    # canonical edge list WITHOUT self loops (self term added on-device)
    src_c = (src // BLK_RAW) * BLK + 1 + (src % BLK_RAW)
    dst_c = (dst // BLK_RAW) * BLK + 1 + (dst % BLK_RAW)
    src_p = phys(src_c)

    per_core = []
    for r in range(N_CORES):
        lo, hi = r * BLK, (r + 1) * BLK
        m = (dst_c >= lo) & (dst_c < hi)
        s_r = src_p[m]
        d_r = dst_c[m] - lo
        passes = []
        for half in (0, 1):
            pm = (s_r >= NROWSA) if half else (s_r < NROWSA)
            s_p = s_r[pm] - half * NROWSA
            d_p = d_r[pm]
            degp = np.bincount(d_p, minlength=BLK)
            perm = np.argsort(degp, kind="stable")       # perm[pos] = local id
            invperm = np.empty(BLK, dtype=np.int64)
            invperm[perm] = np.arange(BLK)
            sorted_deg = degp[perm]
            Kt = sorted_deg.reshape(MT, P).max(axis=1)
            passes.append(dict(s=s_p, d=d_p, invperm=invperm, Kt=Kt,
                               sorted_deg=sorted_deg))
        per_core.append(passes)

    KAg = np.zeros(MT, dtype=np.int64)
    KBg = np.zeros(MT, dtype=np.int64)
    for r in range(N_CORES):
        KAg = np.maximum(KAg, per_core[r][0]["Kt"])
        KBg = np.maximum(KBg, per_core[r][1]["Kt"])
    WA, WB = int(KAg.sum()), int(KBg.sum())
    offA = np.concatenate([[0], np.cumsum(KAg)[:-1]])
    offB = np.concatenate([[0], np.cumsum(KBg)[:-1]])

    def build_grid(info, Kg, off, zrow):
        sumK = int(Kg.sum())
        grid = np.full((sumK, P), zrow, dtype=np.int64)
        s, d, invperm = info["s"], info["d"], info["invperm"]
        pos = invperm[d]
        order = np.argsort(pos, kind="stable")
        pos_s = pos[order]
        s_s = s[order]
        counts = np.bincount(pos_s, minlength=BLK)
        starts = np.concatenate([[0], np.cumsum(counts)[:-1]])
        k = np.arange(len(pos_s)) - starts[pos_s]
        tile_i = pos_s // P
        lane = pos_s % P
        grid[off[tile_i] + k, lane] = s_s
        return grid

    in_maps = []
    xt_blocks = []
    for r in range(N_CORES):
        lo = r * BLK_RAW
        hi = min(N, (r + 1) * BLK_RAW)
        xb = np.zeros((BLK, IN), dtype=np.float32)
        if hi > lo:
            xb[1 : 1 + hi - lo] = x[lo:hi]
        xt_blocks.append(np.ascontiguousarray(xb.T).astype(BF16))

    b1r = np.tile(np.asarray(b1, np.float32)[None, :], (P, 1))
    b2r = np.tile(np.asarray(b2, np.float32)[None, :], (P, 1))
    bfcr = np.tile(np.asarray(bfc, np.float32)[None, :], (P, 1))
    w1 = np.asarray(W1, np.float32).astype(BF16)
    w2 = np.asarray(W2, np.float32).astype(BF16)
    wfc = np.asarray(Wfc, np.float32).astype(BF16)

    for r in range(N_CORES):
        pa, pb = per_core[r]
        gridA = build_grid(pa, KAg, offA, ZROW_A)
        gridB = build_grid(pb, KBg, offB, ZROW_B)
        dv = dinv_c[r * BLK : (r + 1) * BLK].reshape(MT, P).T.copy()  # [P, MT]
        in_maps.append({
            "xt": xt_blocks[r],
            "w1": w1, "w2": w2, "wfc": wfc,
            "b1r": b1r, "b2r": b2r, "bfcr": bfcr,
            "dinv": np.ascontiguousarray(dv),
            "idxa": np.ascontiguousarray(_wrap16(gridA.reshape(-1))),
            "idxb": np.ascontiguousarray(_wrap16(gridB.reshape(-1))),
            "mapa": np.ascontiguousarray(_wrap16(pa["invperm"])),
            "mapb": np.ascontiguousarray(_wrap16(pb["invperm"])),
        })

    meta = dict(N=N, IN=IN, HID=HID, CLS=CLS, BLK=BLK, BLK_RAW=BLK_RAW,
                NPAD=NPAD, MT=MT, MTA=MTA, NROWSA=NROWSA, NROWSB=NROWSB,
                KA=tuple(int(k) for k in KAg), KB=tuple(int(k) for k in KBg))
    return in_maps, meta


# ----------------------------------------------------------------------------
# Device graph
# ----------------------------------------------------------------------------

def _tree_reduce_into(nc, g, n, out_ap, eng=None, order_fn=None):
    """Sum g[:, :n, :] slabs; final level writes into out_ap."""
    e = eng if eng is not None else nc.any

    def emit(fn, *args):
        inst = fn(*args)
        if order_fn is not None:
            order_fn(inst)
        return inst

    if n == 1:
        emit(e.tensor_copy, out_ap, g[:, 0, :])
        return
    while n > 2:
        if n % 2 == 1:
            emit(e.tensor_add, g[:, 0, :], g[:, 0, :], g[:, n - 1, :])
            n -= 1
            if n == 2:
                break
        h = n // 2
        emit(e.tensor_add, g[:, :h, :], g[:, :h, :], g[:, h : 2 * h, :])
        n = h
    emit(e.tensor_add, out_ap, g[:, 0, :], g[:, 1, :])


def _build(meta, stage="full"):
    IN, HID, CLS = meta["IN"], meta["HID"], meta["CLS"]
    BLK, NPAD, MT = meta["BLK"], meta["NPAD"], meta["MT"]
    MTA = meta["MTA"]
    NROWSA, NROWSB = meta["NROWSA"], meta["NROWSB"]
    KA, KB = meta["KA"], meta["KB"]
    WA, WB = sum(KA), sum(KB)
    KC = IN // P

    tdt = dt.bfloat16 if TBL16 else dt.float32

    nc = bacc.Bacc("TRN2", target_bir_lowering=False, debug=False,
                   num_devices=N_CORES, num_swdge_queues=NQ)

    xt = nc.dram_tensor("xt", [IN, BLK], dt.bfloat16, kind="ExternalInput")
    w1 = nc.dram_tensor("w1", [IN, HID], dt.bfloat16, kind="ExternalInput")
    w2 = nc.dram_tensor("w2", [HID, HID], dt.bfloat16, kind="ExternalInput")
    wfc = nc.dram_tensor("wfc", [HID, CLS], dt.bfloat16, kind="ExternalInput")
    b1r = nc.dram_tensor("b1r", [P, HID], dt.float32, kind="ExternalInput")
    b2r = nc.dram_tensor("b2r", [P, HID], dt.float32, kind="ExternalInput")
    bfcr = nc.dram_tensor("bfcr", [P, CLS], dt.float32, kind="ExternalInput")
    dinv = nc.dram_tensor("dinv", [P, MT], dt.float32, kind="ExternalInput")
    idxa = nc.dram_tensor("idxa", [P, WA * 8], dt.int16, kind="ExternalInput")
    idxb = nc.dram_tensor("idxb", [P, WB * 8], dt.int16, kind="ExternalInput")
    mapa = nc.dram_tensor("mapa", [P, BLK // 16], dt.int16, kind="ExternalInput")
    mapb = nc.dram_tensor("mapb", [P, BLK // 16], dt.int16, kind="ExternalInput")
    out = nc.dram_tensor("out", [BLK, CLS], dt.float32, kind="ExternalOutput")

    NSEM_PQ = -(-max(PREPN, 16) // NQ)  # sem slots per queue
    with tile.TileContext(nc) as tc:
        psems = [[nc.alloc_semaphore(f"pp{q}_{i}") for i in range(NSEM_PQ)]
                 for q in range(NQ)]
        ptarget = [[0] * NSEM_PQ for _ in range(NQ)]
        prr = [0] * NQ
        with (
            tc.tile_pool(name="const", bufs=1) as cpool,
            tc.tile_pool(name="idx", bufs=1) as ipool,
            tc.tile_pool(name="part", bufs=2) as partpool,
            tc.tile_pool(name="hs", bufs=1) as hspool,
            tc.tile_pool(name="hnew", bufs=1) as hnewpool,
            tc.tile_pool(name="acc", bufs=1) as accpool,
            tc.tile_pool(name="outsb", bufs=1) as outpool,
            tc.tile_pool(name="xload", bufs=3) as xpool,
            tc.tile_pool(name="gbuf", bufs=GBUFS) as gpool,
            tc.tile_pool(name="pbuf", bufs=max(PREPN, 1)) as ppool,
            tc.tile_pool(name="tok", bufs=4) as tokpool,
            tc.tile_pool(name="lhsT", bufs=3) as tpool,
            tc.tile_pool(name="ps", bufs=3, space="PSUM") as pspool,
            tc.tile_pool(name="pst", bufs=2, space="PSUM") as pstpool,
            tc.tile_pool(name="dram", bufs=1, space="DRAM") as dpool,
        ):
            # ---- constants ----
            w1sb = cpool.tile([P, KC, HID], dt.bfloat16, tag="w1")
            nc.sync.dma_start(out=w1sb[:], in_=w1[:].rearrange("(c k) h -> k c h", k=P))
            w2sb = cpool.tile([P, HID], dt.bfloat16, tag="w2")
            nc.sync.dma_start(out=w2sb[:], in_=w2[:])
            wfcsb = cpool.tile([P, CLS], dt.bfloat16, tag="wfc")
            nc.sync.dma_start(out=wfcsb[:], in_=wfc[:])
            b1sb = cpool.tile([P, HID], dt.float32, tag="b1")
            nc.sync.dma_start(out=b1sb[:], in_=b1r[:])
            b2sb = cpool.tile([P, HID], dt.float32, tag="b2")
            nc.sync.dma_start(out=b2sb[:], in_=b2r[:])
            bfcsb = cpool.tile([P, CLS], dt.float32, tag="bfc")
            nc.sync.dma_start(out=bfcsb[:], in_=bfcr[:])
            dvsb = cpool.tile([P, MT], dt.float32, tag="dinv")
            nc.sync.dma_start(out=dvsb[:], in_=dinv[:])
            ident = cpool.tile([P, P], dt.float32, tag="ident")
            make_identity(nc, ident[:])

            idxasb = ipool.tile([P, WA * 8], dt.int16, tag="idxa")
            nc.sync.dma_start(out=idxasb[:], in_=idxa[:])
            idxbsb = ipool.tile([P, WB * 8], dt.int16, tag="idxb")
            nc.sync.dma_start(out=idxbsb[:], in_=idxb[:])
            mapasb = ipool.tile([P, BLK // 16], dt.int16, tag="mapa")
            nc.sync.dma_start(out=mapasb[:], in_=mapa[:])
            mapbsb = ipool.tile([P, BLK // 16], dt.int16, tag="mapb")
            nc.sync.dma_start(out=mapbsb[:], in_=mapb[:])

            self_q = [0]   # rotating SWDGE queue assignment
            qload = [0] * NQ  # cumulative idx load per queue

            def pick_q(n_idx, allowed=None):
                cand = allowed if allowed else range(NQ)
                q = min(cand, key=lambda j: qload[j])
                qload[q] += n_idx
                return q
            # Tile may reorder Pool instructions; the SWDGE FIFO forbids a
            # self-triggered (gen0) op behind untriggered preps. Keep explicit
            # per-queue nosync ordering: gen0 after last trigger, prep after
            # last gen0/trigger.
            last_gen0 = [None] * NQ
            last_trig = [None] * NQ

            from concourse.bass import InstructionNameOrderedSet as _INOS

            def order_after(inst, *names):
                deps = _INOS()
                for n in names:
                    if n:
                        deps.add(n)
                if len(deps):
                    inst.ins.add_nosync_dependencies_from(deps)
                return inst.ins.name

            def shadow_of(handle, name, shape, dtype, addr_space):
                """Same-address alias with a fresh tensor name: reads through
                it carry no Tile dependency on the real tensor's writers.
                Used for prepare_only desc-gen ahead of the data landing
                (ordering is restored at trigger time via signals_writable)."""
                if not SHADOW:
                    return handle  # sim mode: name-based interp can't alias
                sh = nc.dram_tensor(name, shape, dtype, addr_space=addr_space)
                src = nc.lookup_mls(handle).memorylocations[0]
                dst = nc.lookup_mls(sh).memorylocations[0]
                dst.addr = src.addr
                return sh

            def next_sem(q):
                i = prr[q] % NSEM_PQ
                prr[q] += 1
                ptarget[q][i] += 16
                return psems[q][i], ptarget[q][i]

            h_prev = None
            for layer in (1, 2):
                # ---- dense transform + pre-scale, AG kicked per half ----
                hs = hspool.tile([P, MT, HID], tdt, tag="hs")
                tables = []
                tokens = []
                agb_name = [None]

                def mm_tile(m):
                    ps = pspool.tile([P, HID], dt.float32, tag="mm")
                    if layer == 1:
                        xm = xpool.tile([P, KC, P], dt.bfloat16, tag="x")
                        nc.sync.dma_start(
                            out=xm[:],
                            in_=xt[:].rearrange("(c k) m -> k c m", k=P)[
                                :, :, m * P : (m + 1) * P
                            ],
                        )
                        for c in range(KC):
                            nc.tensor.matmul(
                                ps[:], xm[:, c, :], w1sb[:, c, :],
                                start=(c == 0), stop=(c == KC - 1),
                            )
                    else:
                        pst = pstpool.tile([P, P], dt.float32, tag="tr")
                        nc.tensor.transpose(pst[:], h_prev[:, m, :], ident[:])
                        hT = tpool.tile([P, P], dt.bfloat16, tag="hT")
                        nc.any.tensor_copy(hT[:], pst[:])
                        nc.tensor.matmul(ps[:], hT[:], w2sb[:],
                                         start=True, stop=True)
                    nc.vector.tensor_scalar_mul(hs[:, m, :], ps[:],
                                                dvsb[:, m : m + 1])

                for hseg, (t0, t1, nrows) in enumerate(
                    ((0, MTA, NROWSA), (MTA, MT, NROWSB))
                ):
                    for m in range(t0, t1):
                        mm_tile(m)
                    agin = dpool.tile([(t1 - t0) * P, HID], tdt,
                                      tag=f"agin{layer}{hseg}")
                    nc.sync.dma_start(
                        out=agin[:].rearrange("(t p) h -> p t h", p=P),
                        in_=hs[:, t0:t1, :],
                    )
                    tbl = nc.dram_tensor(f"table{layer}{hseg}", [nrows, HID],
                                         tdt, addr_space="Shared")
                    agi = nc.gpsimd.collective_compute(
                        "AllGather",
                        mybir.AluOpType.bypass,
                        replica_groups=[list(range(N_CORES))],
                        ins=[agin[:].opt()],
                        outs=[tbl[:].opt()],
                    )
                    agb_name[0] = agi.ins.name
                    tables.append(
                        (tbl, shadow_of(tbl, f"tblsh{layer}{hseg}",
                                        [nrows, HID], tdt, "Shared")))

                # tokens last: a token DMA waits its AllGather, and the SP
                # stream is in-order — emitting it mid-transform would stall
                # the half-B xloads behind AG_A.
                for hseg, (tbl, _sh) in enumerate(tables):
                    tok = tokpool.tile([P, 2], tdt, tag=f"tok{hseg}")
                    nc.sync.dma_start(
                        out=tok[:, :1],
                        in_=tbl[:].rearrange("(t p) h -> p t h", p=P)[:, :1, :1],
                    )
                    tokens.append(tok)

                # ---- gather + reduce, two passes by source half ----
                # The first PREPN calls of pass A are PREPARE_ONLY so their
                # desc-gen overlaps the transform + AllGather above.
                pdrams = []
                pstate = dict(left=PREPN, queues=set(), pending=False,
                              bq=[0] * NQ)

                def flush_triggers(tok_ap):
                    if pstate["pending"]:
                        for tq in sorted(pstate["queues"]):
                            ti = nc.gpsimd.trigger_dma(
                                count=None, queue_num=tq,
                                signals_writable=[tok_ap[:]],
                            )
                            last_trig[tq] = ti.ins.name
                        pstate["queues"].clear()
                        pstate["pending"] = False
                        pstate["bq"] = [0] * NQ

                def emit_call(g, s0, kc, isb, o0, tview, shview, use_prep,
                              tok_ap, waits):
                    if use_prep:
                        allowed = [j for j in range(NQ)
                                   if pstate["bq"][j] < NSEM_PQ]
                        q = pick_q(kc, allowed)
                        pstate["bq"][q] += 1
                        sem, tgt = next_sem(q)
                        pi = nc.gpsimd.dma_gather(
                            out_ap=g[:, s0 : s0 + kc, :], in_ap=shview,
                            idxs_ap=isb[:, o0 * 8 : (o0 + kc) * 8],
                            num_idxs=kc * P, num_idxs_reg=kc * P,
                            elem_size=HID, queue_num=q,
                            prepare_only=True, sem=sem,
                        )
                        order_after(pi, last_gen0[q], last_trig[q])
                        waits.append((sem, tgt))
                        pstate["queues"].add(q)
                        pstate["left"] -= 1
                        pstate["pending"] = True
                    else:
                        q = pick_q(kc)
                        flush_triggers(tok_ap)
                        gi = nc.gpsimd.dma_gather(
                            out_ap=g[:, s0 : s0 + kc, :], in_ap=tview,
                            idxs_ap=isb[:, o0 * 8 : (o0 + kc) * 8],
                            num_idxs=kc * P, num_idxs_reg=kc * P,
                            elem_size=HID, queue_num=q,
                        )
                        last_gen0[q] = order_after(gi, last_trig[q])

                def emit_group(part, t, K, g0, first, isb, off, tview, shview,
                               use_prep, tok_ap, anchor):
                    cap = CHUNK if use_prep else SCAP
                    Kg = min(cap, K - g0)
                    if use_prep:
                        gt = ppool.tile([P, CHUNK, HID], tdt, tag="p")
                    else:
                        gt = gpool.tile([P, SCAP, HID], tdt, tag="g")
                    g = gt[:]
                    waits = []
                    s0 = 0
                    while s0 < Kg:
                        kc = min(CHUNK, Kg - s0)
                        emit_call(g, s0, kc, isb, off + g0 + s0, tview, shview,
                                  use_prep, tok_ap, waits)
                        s0 += kc
                    eng = nc.vector if waits else nc.any
                    wait_names = []
                    for sem, tgt in waits:
                        wi = nc.vector.wait_ge(sem, tgt)
                        # anchor: the scheduler must not hoist this engine
                        # wait above the work feeding the trigger's deps
                        wait_names.append(order_after(wi, anchor))
                    ofn = (lambda i: order_after(i, *wait_names)) \
                        if wait_names else None
                    if first:
                        _tree_reduce_into(nc, g, Kg, part[:, t, :], eng=eng,
                                          order_fn=ofn)
                    else:
                        tmp = tpool.tile([P, P], tdt, tag="gtmp")
                        _tree_reduce_into(nc, g, Kg, tmp[:, :HID], eng=eng,
                                          order_fn=ofn)
                        ai = eng.tensor_add(part[:, t, :], part[:, t, :],
                                            tmp[:, :HID])
                        if ofn:
                            ofn(ai)
                    return Kg

                def emit_pass(half, Ks, isb, anchor):
                    part = partpool.tile([P, MT, HID], tdt, tag="part")
                    tview = tables[half][0][:]
                    shview = tables[half][1][:]
                    tok_ap = tokens[half]
                    offs = [0] * MT
                    for t in range(1, MT):
                        offs[t] = offs[t - 1] + Ks[t - 1]
                    for t in range(MT):
                        K = Ks[t]
                        if K == 0:
                            nc.vector.memset(part[:, t, :], 0.0)
                            continue
                        off = offs[t]
                        g0 = 0
                        first = True
                        while g0 < K:
                            use_prep = half == 0 and pstate["left"] > 0
                            g0 += emit_group(part, t, K, g0, first, isb, off,
                                             tview, shview, use_prep, tok_ap,
                                             anchor)
                            first = False
                    flush_triggers(tok_ap)
                    pd = nc.dram_tensor(f"pd{layer}{half}", [BLK, HID], tdt)
                    pdw = nc.sync.dma_start(
                        out=pd[:].rearrange("(t p) h -> p t h", p=P),
                        in_=part[:],
                    )
                    pdsh = shadow_of(pd, f"pdsh{layer}{half}", [BLK, HID],
                                     tdt, "Local")
                    pdrams.append((pd, pdsh, pdw.ins.name))

                ag_anchor = agb_name[0]
                emit_pass(0, KA, idxasb, ag_anchor)
                emit_pass(1, KB, idxbsb, ag_anchor)

                # ---- canonicalize (prepped) + merge + bias/relu ----
                accA = accpool.tile([P, MT, HID], tdt, tag="accA")
                accB = accpool.tile([P, MT, HID], tdt, tag="accB")
                canon_waits = {0: [], 1: []}
                pd_anchors = (pdrams[0][2], pdrams[1][2])
                for half, (dst, pdt, mapsb, ttag) in enumerate(
                    ((accA, pdrams[0], mapasb, "ptokA"),
                     (accB, pdrams[1], mapbsb, "ptokB"))
                ):
                    canon_queues = set()
                    canon_bq = [0] * NQ
                    for c0 in range(0, MT, CHUNK):
                        cc = min(CHUNK, MT - c0)
                        allowed = [j for j in range(NQ)
                                   if canon_bq[j] < NSEM_PQ]
                        q = pick_q(cc, allowed)
                        canon_bq[q] += 1
                        sem, tgt = next_sem(q)
                        pi = nc.gpsimd.dma_gather(
                            out_ap=dst[:, c0 : c0 + cc, :], in_ap=pdt[1][:],
                            idxs_ap=mapsb[:, c0 * 8 : (c0 + cc) * 8],
                            num_idxs=cc * P, num_idxs_reg=cc * P,
                            elem_size=HID, queue_num=q,
                            prepare_only=True, sem=sem,
                        )
                        order_after(pi, last_gen0[q], last_trig[q])
                        canon_queues.add(q)
                        canon_waits[half].append((c0, c0 + cc, sem, tgt))
                    ptok = tokpool.tile([P, 2], tdt, tag=ttag)
                    nc.sync.dma_start(
                        out=ptok[:, :1],
                        in_=pdt[0][:].rearrange("(t p) h -> p t h", p=P)[
                            :, :1, :1],
                    )
                    for tq in sorted(canon_queues):
                        ti = nc.gpsimd.trigger_dma(
                            count=None, queue_num=tq,
                            signals_writable=[ptok[:]],
                        )
                        last_trig[tq] = ti.ins.name

                hnew = hnewpool.tile([P, MT, HID], dt.float32, tag="hnew")
                bsb = b1sb if layer == 1 else b2sb
                # quarter-width post ops so downstream per-tile consumers
                # (transposes / next matmul / fc) can start early
                QCH = max(1, (MT + 6) // 7)
                done_sems = {}
                for c0 in range(0, MT, QCH):
                    c1 = min(MT, c0 + QCH)
                    sl = slice(c0, c1)
                    w = c1 - c0
                    cw_names = []
                    for half in (0, 1):
                        for (a0, a1, sem, tgt) in canon_waits[half]:
                            if a0 < c1 and a1 > c0:
                                if (sem, tgt) in done_sems:
                                    cw_names.append(done_sems[(sem, tgt)])
                                else:
                                    wi = nc.vector.wait_ge(sem, tgt)
                                    nm = order_after(wi, *pd_anchors)
                                    done_sems[(sem, tgt)] = nm
                                    cw_names.append(nm)
                    mi = nc.vector.tensor_add(hnew[:, sl, :], accA[:, sl, :],
                                              accB[:, sl, :])
                    order_after(mi, *cw_names)
                    nc.vector.tensor_add(hnew[:, sl, :], hnew[:, sl, :],
                                         hs[:, sl, :])  # self-loop term
                    dv3 = dvsb[:, sl].to_broadcast([P, w, HID])
                    nc.vector.tensor_tensor(hnew[:, sl, :], hnew[:, sl, :],
                                            dv3, op=mybir.AluOpType.mult)
                    b3 = bsb[:].rearrange("p (o h) -> p o h", o=1).to_broadcast(
                        [P, w, HID])
                    nc.vector.tensor_tensor(hnew[:, sl, :], hnew[:, sl, :],
                                            b3, op=mybir.AluOpType.add)
                    nc.scalar.activation(hnew[:, sl, :], hnew[:, sl, :],
                                         mybir.ActivationFunctionType.Relu)
                h_prev = hnew

            # ---- classifier ----
            outsb = outpool.tile([P, MT, CLS], dt.float32, tag="outsb")
            for m in range(MT):
                pst = pstpool.tile([P, P], dt.float32, tag="tr")
                nc.tensor.transpose(pst[:], h_prev[:, m, :], ident[:])
                hT = tpool.tile([P, P], dt.bfloat16, tag="hT")
                nc.any.tensor_copy(hT[:], pst[:])
                ps2 = pspool.tile([P, CLS], dt.float32, tag="mm2")
                nc.tensor.matmul(ps2[:], hT[:], wfcsb[:], start=True, stop=True)
                nc.vector.tensor_add(outsb[:, m, :], ps2[:], bfcsb[:])
                if m % 12 == 11 or m == MT - 1:
                    m0 = (m // 12) * 12
                    nc.sync.dma_start(
                        out=out[:].rearrange("(t p) c -> p t c", p=P)[
                            :, m0 : m + 1, :
                        ],
                        in_=outsb[:, m0 : m + 1, :],
                    )

    # The tile pass emits the prepare_only DMASW pre-bumps (InstIncSwdgeSem)
    # with queue_num=0 regardless of the prep's queue; the ucode's per-queue
    # ring bookkeeping (and the sim's queue locks) need the real queue.
    import concourse.bass_isa as _bisa
    lanes_per_q = max(1, 8 // NQ)
    for _fn in nc.m.functions:
        for _blk in _fn.blocks:
            for _inst in _blk.instructions:
                if isinstance(_inst, _bisa.InstIncSwdgeSem) and \
                        _inst._mode == "add":
                    for _n, _v in zip(_inst._sem_names, _inst._sem_values):
                        if _v and _n.startswith("DMASW"):
                            lane = int(_n[5:].split("_")[0])
                            _inst.queue_num = lane // lanes_per_q
                            break

    nc.compile()
    return nc


# ----------------------------------------------------------------------------
# Entry point
# ----------------------------------------------------------------------------

_CACHE = {}


def _get_graph(meta):
    key = (meta["IN"], meta["HID"], meta["CLS"], meta["BLK"], meta["NPAD"],
           meta["KA"], meta["KB"])
    if key not in _CACHE:
        _CACHE[key] = _build(meta)
    return _CACHE[key]


def kernel(x, edge_index, W1, b1, W2, b2, Wfc, bfc, _want_profile=False,
           _stage="full"):
    x = np.asarray(x, dtype=np.float32)
    in_maps, meta = _preprocess(np.asarray(x), np.asarray(edge_index),
                                np.asarray(W1), np.asarray(b1),
                                np.asarray(W2), np.asarray(b2),
                                np.asarray(Wfc), np.asarray(bfc))
    nc = _get_graph(meta)
    res = run_bass_kernel_spmd(nc, in_maps, core_ids=list(range(N_CORES)),
                               trace=_want_profile)
    N, CLS = meta["N"], meta["CLS"]
    BLK_RAW = meta["BLK_RAW"]
    full = np.empty((N, CLS), dtype=np.float32)
    for r in range(N_CORES):
        lo = r * BLK_RAW
        hi = min(N, (r + 1) * BLK_RAW)
        if hi > lo:
            full[lo:hi] = res.results[r]["out"][1 : 1 + hi - lo]
    if _want_profile:
        return full, res
    return full


# revision 41
# speedup vs baseline: 1.0068x; 1.0068x over previous
"""Trainium2 Bass kernel for a 2-layer GCN + linear classifier (PyG GCNConv style).

Self-contained: hardcodes the 8-core sharding strategy; all graph/index
preprocessing is host-side numpy, all FLOPs on x run on device.

Sharding: nodes are split into 8 contiguous canonical blocks (one per core,
padded to a multiple of 128; slot 0 of each block is a guaranteed-zero pad
row). Per GCN layer each core computes its block's dense transform (bf16
PE matmuls, f32 PSUM), pre-scales rows by dinv, and two AllGathers (kicked
per table half, interleaved with the transform) materialize the full bf16
node table in every core's HBM. Each core aggregates its own destinations'
in-edges with gpsimd dma_gather (256B bf16 rows, <=1024 indices per
instruction — ucode descriptor-ring limit — over 4 SWDGE queues) plus
DVE/ACT tree reductions, in two passes split by physical table half so
gather indices fit int16. Destinations are degree-sorted per (core, pass)
to minimize slab padding; a small canonicalization gather restores node
order via a bf16 partial-sum round trip through DRAM.

Pool-engine descriptor generation (~6.5ns/idx per queue, 4 queues max) is
the bottleneck. To hide the serial transform/AllGather windows, the first
gather calls of each layer's pass A — and the canonicalization gathers —
are issued as PREPARE_ONLY preps (desc-gen runs before the table data
exists; deps defer to trigger_dma). Their DMA completion is gated by
per-call user semaphores + explicit consumer wait_ge (Tile's automatic
DMASW tracking pre-bumps lane sems at prep time, so it cannot order
consumers of prepped gathers).
"""

import os
import sys
import types

import numpy as np


def _setup_env():
    if "/opt/trn_rl_repo" not in sys.path:
        sys.path.insert(0, "/opt/trn_rl_repo")
    if "antenv.axon_hooks" not in sys.modules:
        try:
            from trn_agent_boot.trn_boot import _ntff_profile_via_ctypes

            _hook = _ntff_profile_via_ctypes("/opt/axon/libaxon_pjrt.so")
        except Exception:
            _hook = None
        _mod = types.ModuleType("antenv.axon_hooks")
        _mod.get_axon_ntff_profile_hook = lambda: _hook
        _mod.set_axon_ntff_profile_hook = lambda h: None
        sys.modules["antenv.axon_hooks"] = _mod


_setup_env()

import ml_dtypes  # noqa: E402
from concourse import bacc, bass, mybir, tile  # noqa: E402
import concourse.bass_utils as bass_utils  # noqa: E402
from concourse.bass_utils import run_bass_kernel_spmd  # noqa: E402
from concourse.masks import make_identity  # noqa: E402

bass_utils.upload_artifacts = lambda tmpdir: tmpdir

# --- queue-aware DMASW semaphore lane assignment -----------------------------
# Tile assigns Pool-engine DMA instructions to the 8 DMASW semaphore lanes
# round-robin in *scheduled* order, but each lane gets locked to the SWDGE
# queue of the first instruction using it. With multi-queue dma_gather this
# races; pin each queue to its own lane subset instead.
import concourse.tile_sem_assignment as _tsa  # noqa: E402
from concourse.bass_isa import UserSyncedRemoteDMADescs as _URD  # noqa: E402
from concourse.tile_sem_assignment import DMAInst as _DMAInst  # noqa: E402

_orig_assign_tick = _tsa.TileClockTick._assign_tick


def _queue_aware_assign_tick(self, inst):
    if (
        isinstance(inst, _DMAInst)
        and not isinstance(inst, _URD)
        and inst.engine == mybir.EngineType.Pool
    ):
        q = getattr(inst, "queue_num", 0) or 0
        lanes = max(1, self.swdge_sem_count // NQ)
        rot = self.__dict__.setdefault("_q_lane_rot", {})
        r = rot.get(q, 0)
        self.next_sw_dma_idx = (q * lanes + r) % self.swdge_sem_count
        rot[q] = (r + 1) % lanes
    return _orig_assign_tick(self, inst)


_tsa.TileClockTick._assign_tick = _queue_aware_assign_tick
# -----------------------------------------------------------------------------

N_CORES = 8
P = 128
CHUNK = 8   # max gather slabs (of 128 rows) per dma_gather instruction
            # (hard ucode limit: 1024 idxs per instruction)
NQ = int(os.environ.get("KNQ", "4"))  # SWDGE queues (desc-gen parallelism)
GBUFS = int(os.environ.get("KGBUFS", "7"))
SCAP = int(os.environ.get("KSCAP", "24"))  # max staging slabs per group
PREPN = int(os.environ.get("KPREPN", "16"))  # prepare_only calls per boundary
TBL16 = os.environ.get("KTBL16", "1") == "1"  # bf16 gather table
SHADOW = os.environ.get("KSHADOW", "1") == "1"  # alias tables for early desc-gen

dt = mybir.dt
BF16 = ml_dtypes.bfloat16


# ----------------------------------------------------------------------------
# Host-side preprocessing
# ----------------------------------------------------------------------------

def _wrap16(flat: np.ndarray) -> np.ndarray:
    """Lay out an index list in dma_gather's [128, n/16] wrapped format."""
    n = flat.shape[0]
    assert n % 16 == 0
    w = flat.reshape(n // 16, 16).T.astype(np.int16)  # [16, n//16]
    return np.tile(w, (8, 1))  # replicate across the 8 groups of 16 partitions


def _preprocess(x, edge_index, W1, b1, W2, b2, Wfc, bfc):
    N, IN = x.shape
    HID = W1.shape[1]
    CLS = Wfc.shape[1]
    E = edge_index.shape[1]
    assert IN % P == 0 and HID == P

    BLK_RAW = -(-N // N_CORES)            # nodes per core before padding
    BLK = -(-BLK_RAW // P) * P            # padded block size
    assert BLK_RAW + 1 <= BLK, "need pad slots per block"
    NPAD = N_CORES * BLK
    MT = BLK // P
    MTA = MT // 2                         # tiles per block in table half A
    HA = MTA * P                          # rows per block in half A
    HB = BLK - HA
    NROWSA = N_CORES * HA                 # physical half-A table rows
    NROWSB = N_CORES * HB
    assert NROWSA < 32768 and NROWSB < 32768

    src = edge_index[0].astype(np.int64)
    dst = edge_index[1].astype(np.int64)

    deg = np.bincount(dst, minlength=N).astype(np.float64) + 1.0
    dinv = (1.0 / np.sqrt(deg)).astype(np.float32)
    dinv_c = np.zeros(NPAD, dtype=np.float32)
    all_ids = np.arange(N, dtype=np.int64)
    # block-local slot: j=0 reserved as a guaranteed-zero pad row (half A),
    # reals at j in [1, BLK_RAW], remaining pads at the tail (half B).
    canon = (all_ids // BLK_RAW) * BLK + 1 + (all_ids % BLK_RAW)
    dinv_c[canon] = dinv

    def phys(c):
        r = c // BLK
        j = c % BLK
        return np.where(j < HA, r * HA + j, NROWSA + r * HB + (j - HA))

    ZROW_A = 0                              # block 0, j=0
    assert BLK_RAW + 1 < BLK, "need a tail pad slot per block"
    assert BLK_RAW + 1 >= HA, "tail pads must land in half B"
    ZROW_B = int(phys(np.array([BLK_RAW + 1]))[0] - NROWSA)

    # canonical edge list WITHOUT self loops (self term added on-device)
    src_c = (src // BLK_RAW) * BLK + 1 + (src % BLK_RAW)
    dst_c = (dst // BLK_RAW) * BLK + 1 + (dst % BLK_RAW)
    src_p = phys(src_c)

    per_core = []
    for r in range(N_CORES):
        lo, hi = r * BLK, (r + 1) * BLK
        m = (dst_c >= lo) & (dst_c < hi)
        s_r = src_p[m]
        d_r = dst_c[m] - lo
        passes = []
        for half in (0, 1):
            pm = (s_r >= NROWSA) if half else (s_r < NROWSA)
            s_p = s_r[pm] - half * NROWSA
            d_p = d_r[pm]
            degp = np.bincount(d_p, minlength=BLK)
            perm = np.argsort(degp, kind="stable")       # perm[pos] = local id
            invperm = np.empty(BLK, dtype=np.int64)
            invperm[perm] = np.arange(BLK)
            sorted_deg = degp[perm]
            Kt = sorted_deg.reshape(MT, P).max(axis=1)
            passes.append(dict(s=s_p, d=d_p, invperm=invperm, Kt=Kt,
                               sorted_deg=sorted_deg))
        per_core.append(passes)

    KAg = np.zeros(MT, dtype=np.int64)
    KBg = np.zeros(MT, dtype=np.int64)
    for r in range(N_CORES):
        KAg = np.maximum(KAg, per_core[r][0]["Kt"])
        KBg = np.maximum(KBg, per_core[r][1]["Kt"])
    WA, WB = int(KAg.sum()), int(KBg.sum())
    offA = np.concatenate([[0], np.cumsum(KAg)[:-1]])
    offB = np.concatenate([[0], np.cumsum(KBg)[:-1]])

    def build_grid(info, Kg, off, zrow):
        sumK = int(Kg.sum())
        grid = np.full((sumK, P), zrow, dtype=np.int64)
        s, d, invperm = info["s"], info["d"], info["invperm"]
        pos = invperm[d]
        order = np.argsort(pos, kind="stable")
        pos_s = pos[order]
        s_s = s[order]
        counts = np.bincount(pos_s, minlength=BLK)
        starts = np.concatenate([[0], np.cumsum(counts)[:-1]])
        k = np.arange(len(pos_s)) - starts[pos_s]
        tile_i = pos_s // P
        lane = pos_s % P
        grid[off[tile_i] + k, lane] = s_s
        return grid

    in_maps = []
    xt_blocks = []
    for r in range(N_CORES):
        lo = r * BLK_RAW
        hi = min(N, (r + 1) * BLK_RAW)
        xb = np.zeros((BLK, IN), dtype=np.float32)
        if hi > lo:
            xb[1 : 1 + hi - lo] = x[lo:hi]
        xt_blocks.append(np.ascontiguousarray(xb.T).astype(BF16))

    b1r = np.tile(np.asarray(b1, np.float32)[None, :], (P, 1))
    b2r = np.tile(np.asarray(b2, np.float32)[None, :], (P, 1))
    bfcr = np.tile(np.asarray(bfc, np.float32)[None, :], (P, 1))
    w1 = np.asarray(W1, np.float32).astype(BF16)
    w2 = np.asarray(W2, np.float32).astype(BF16)
    wfc = np.asarray(Wfc, np.float32).astype(BF16)

    for r in range(N_CORES):
        pa, pb = per_core[r]
        gridA = build_grid(pa, KAg, offA, ZROW_A)
        gridB = build_grid(pb, KBg, offB, ZROW_B)
        dv = dinv_c[r * BLK : (r + 1) * BLK].reshape(MT, P).T.copy()  # [P, MT]
        in_maps.append({
            "xt": xt_blocks[r],
            "w1": w1, "w2": w2, "wfc": wfc,
            "b1r": b1r, "b2r": b2r, "bfcr": bfcr,
            "dinv": np.ascontiguousarray(dv),
            "idxa": np.ascontiguousarray(_wrap16(gridA.reshape(-1))),
            "idxb": np.ascontiguousarray(_wrap16(gridB.reshape(-1))),
            "mapa": np.ascontiguousarray(_wrap16(pa["invperm"])),
            "mapb": np.ascontiguousarray(_wrap16(pb["invperm"])),
        })

    meta = dict(N=N, IN=IN, HID=HID, CLS=CLS, BLK=BLK, BLK_RAW=BLK_RAW,
                NPAD=NPAD, MT=MT, MTA=MTA, NROWSA=NROWSA, NROWSB=NROWSB,
                KA=tuple(int(k) for k in KAg), KB=tuple(int(k) for k in KBg))
    return in_maps, meta


# ----------------------------------------------------------------------------
# Device graph
# ----------------------------------------------------------------------------

def _tree_reduce_into(nc, g, n, out_ap, eng=None, order_fn=None):
    """Sum g[:, :n, :] slabs; final level writes into out_ap."""
    e = eng if eng is not None else nc.any

    def emit(fn, *args):
        inst = fn(*args)
        if order_fn is not None:
            order_fn(inst)
        return inst

    if n == 1:
        emit(e.tensor_copy, out_ap, g[:, 0, :])
        return
    while n > 2:
        if n % 2 == 1:
            emit(e.tensor_add, g[:, 0, :], g[:, 0, :], g[:, n - 1, :])
            n -= 1
            if n == 2:
                break
        h = n // 2
        emit(e.tensor_add, g[:, :h, :], g[:, :h, :], g[:, h : 2 * h, :])
        n = h
    emit(e.tensor_add, out_ap, g[:, 0, :], g[:, 1, :])


def _build(meta, stage="full"):
    IN, HID, CLS = meta["IN"], meta["HID"], meta["CLS"]
    BLK, NPAD, MT = meta["BLK"], meta["NPAD"], meta["MT"]
    MTA = meta["MTA"]
    NROWSA, NROWSB = meta["NROWSA"], meta["NROWSB"]
    KA, KB = meta["KA"], meta["KB"]
    WA, WB = sum(KA), sum(KB)
    KC = IN // P

    tdt = dt.bfloat16 if TBL16 else dt.float32

    nc = bacc.Bacc("TRN2", target_bir_lowering=False, debug=False,
                   num_devices=N_CORES, num_swdge_queues=NQ)

    xt = nc.dram_tensor("xt", [IN, BLK], dt.bfloat16, kind="ExternalInput")
    w1 = nc.dram_tensor("w1", [IN, HID], dt.bfloat16, kind="ExternalInput")
    w2 = nc.dram_tensor("w2", [HID, HID], dt.bfloat16, kind="ExternalInput")
    wfc = nc.dram_tensor("wfc", [HID, CLS], dt.bfloat16, kind="ExternalInput")
    b1r = nc.dram_tensor("b1r", [P, HID], dt.float32, kind="ExternalInput")
    b2r = nc.dram_tensor("b2r", [P, HID], dt.float32, kind="ExternalInput")
    bfcr = nc.dram_tensor("bfcr", [P, CLS], dt.float32, kind="ExternalInput")
    dinv = nc.dram_tensor("dinv", [P, MT], dt.float32, kind="ExternalInput")
    idxa = nc.dram_tensor("idxa", [P, WA * 8], dt.int16, kind="ExternalInput")
    idxb = nc.dram_tensor("idxb", [P, WB * 8], dt.int16, kind="ExternalInput")
    mapa = nc.dram_tensor("mapa", [P, BLK // 16], dt.int16, kind="ExternalInput")
    mapb = nc.dram_tensor("mapb", [P, BLK // 16], dt.int16, kind="ExternalInput")
    out = nc.dram_tensor("out", [BLK, CLS], dt.float32, kind="ExternalOutput")

    NSEM_PQ = -(-max(PREPN, 16) // NQ)  # sem slots per queue
    with tile.TileContext(nc) as tc:
        psems = [[nc.alloc_semaphore(f"pp{q}_{i}") for i in range(NSEM_PQ)]
                 for q in range(NQ)]
        ptarget = [[0] * NSEM_PQ for _ in range(NQ)]
        prr = [0] * NQ
        with (
            tc.tile_pool(name="const", bufs=1) as cpool,
            tc.tile_pool(name="idx", bufs=1) as ipool,
            tc.tile_pool(name="part", bufs=2) as partpool,
            tc.tile_pool(name="hs", bufs=1) as hspool,
            tc.tile_pool(name="hnew", bufs=1) as hnewpool,
            tc.tile_pool(name="acc", bufs=1) as accpool,
            tc.tile_pool(name="outsb", bufs=1) as outpool,
            tc.tile_pool(name="xload", bufs=3) as xpool,
            tc.tile_pool(name="gbuf", bufs=GBUFS) as gpool,
            tc.tile_pool(name="pbuf", bufs=max(PREPN, 1)) as ppool,
            tc.tile_pool(name="tok", bufs=4) as tokpool,
            tc.tile_pool(name="lhsT", bufs=3) as tpool,
            tc.tile_pool(name="ps", bufs=3, space="PSUM") as pspool,
            tc.tile_pool(name="pst", bufs=2, space="PSUM") as pstpool,
            tc.tile_pool(name="dram", bufs=1, space="DRAM") as dpool,
        ):
            # ---- constants ----
            w1sb = cpool.tile([P, KC, HID], dt.bfloat16, tag="w1")
            nc.sync.dma_start(out=w1sb[:], in_=w1[:].rearrange("(c k) h -> k c h", k=P))
            w2sb = cpool.tile([P, HID], dt.bfloat16, tag="w2")
            nc.sync.dma_start(out=w2sb[:], in_=w2[:])
            wfcsb = cpool.tile([P, CLS], dt.bfloat16, tag="wfc")
            nc.sync.dma_start(out=wfcsb[:], in_=wfc[:])
            b1sb = cpool.tile([P, HID], dt.float32, tag="b1")
            nc.sync.dma_start(out=b1sb[:], in_=b1r[:])
            b2sb = cpool.tile([P, HID], dt.float32, tag="b2")
            nc.sync.dma_start(out=b2sb[:], in_=b2r[:])
            bfcsb = cpool.tile([P, CLS], dt.float32, tag="bfc")
            nc.sync.dma_start(out=bfcsb[:], in_=bfcr[:])
            dvsb = cpool.tile([P, MT], dt.float32, tag="dinv")
            nc.sync.dma_start(out=dvsb[:], in_=dinv[:])
            ident = cpool.tile([P, P], dt.float32, tag="ident")
            make_identity(nc, ident[:])

            idxasb = ipool.tile([P, WA * 8], dt.int16, tag="idxa")
            nc.sync.dma_start(out=idxasb[:], in_=idxa[:])
            idxbsb = ipool.tile([P, WB * 8], dt.int16, tag="idxb")
            nc.sync.dma_start(out=idxbsb[:], in_=idxb[:])
            mapasb = ipool.tile([P, BLK // 16], dt.int16, tag="mapa")
            nc.sync.dma_start(out=mapasb[:], in_=mapa[:])
            mapbsb = ipool.tile([P, BLK // 16], dt.int16, tag="mapb")
            nc.sync.dma_start(out=mapbsb[:], in_=mapb[:])

            self_q = [0]   # rotating SWDGE queue assignment
            qload = [0] * NQ  # cumulative idx load per queue

            def pick_q(n_idx, allowed=None):
                cand = allowed if allowed else range(NQ)
                q = min(cand, key=lambda j: qload[j])
                qload[q] += n_idx
                return q
            # Tile may reorder Pool instructions; the SWDGE FIFO forbids a
            # self-triggered (gen0) op behind untriggered preps. Keep explicit
            # per-queue nosync ordering: gen0 after last trigger, prep after
            # last gen0/trigger.
            last_gen0 = [None] * NQ
            last_trig = [None] * NQ

            from concourse.bass import InstructionNameOrderedSet as _INOS

            def order_after(inst, *names):
                deps = _INOS()
                for n in names:
                    if n:
                        deps.add(n)
                if len(deps):
                    inst.ins.add_nosync_dependencies_from(deps)
                return inst.ins.name

            def shadow_of(handle, name, shape, dtype, addr_space):
                """Same-address alias with a fresh tensor name: reads through
                it carry no Tile dependency on the real tensor's writers.
                Used for prepare_only desc-gen ahead of the data landing
                (ordering is restored at trigger time via signals_writable)."""
                if not SHADOW:
                    return handle  # sim mode: name-based interp can't alias
                sh = nc.dram_tensor(name, shape, dtype, addr_space=addr_space)
                src = nc.lookup_mls(handle).memorylocations[0]
                dst = nc.lookup_mls(sh).memorylocations[0]
                dst.addr = src.addr
                return sh

            def next_sem(q):
                i = prr[q] % NSEM_PQ
                prr[q] += 1
                ptarget[q][i] += 16
                return psems[q][i], ptarget[q][i]

            h_prev = None
            for layer in (1, 2):
                # ---- dense transform + pre-scale, AG kicked per half ----
                hs = hspool.tile([P, MT, HID], tdt, tag="hs")
                tables = []
                tokens = []
                agb_name = [None]

                def mm_tile(m):
                    ps = pspool.tile([P, HID], dt.float32, tag="mm")
                    if layer == 1:
                        xm = xpool.tile([P, KC, P], dt.bfloat16, tag="x")
                        nc.sync.dma_start(
                            out=xm[:],
                            in_=xt[:].rearrange("(c k) m -> k c m", k=P)[
                                :, :, m * P : (m + 1) * P
                            ],
                        )
                        for c in range(KC):
                            nc.tensor.matmul(
                                ps[:], xm[:, c, :], w1sb[:, c, :],
                                start=(c == 0), stop=(c == KC - 1),
                            )
                    else:
                        pst = pstpool.tile([P, P], dt.float32, tag="tr")
                        nc.tensor.transpose(pst[:], h_prev[:, m, :], ident[:])
                        hT = tpool.tile([P, P], dt.bfloat16, tag="hT")
                        nc.any.tensor_copy(hT[:], pst[:])
                        nc.tensor.matmul(ps[:], hT[:], w2sb[:],
                                         start=True, stop=True)
                    nc.vector.tensor_scalar_mul(hs[:, m, :], ps[:],
                                                dvsb[:, m : m + 1])

                for hseg, (t0, t1, nrows) in enumerate(
                    ((0, MTA, NROWSA), (MTA, MT, NROWSB))
                ):
                    for m in range(t0, t1):
                        mm_tile(m)
                    agin = dpool.tile([(t1 - t0) * P, HID], tdt,
                                      tag=f"agin{layer}{hseg}")
                    nc.sync.dma_start(
                        out=agin[:].rearrange("(t p) h -> p t h", p=P),
                        in_=hs[:, t0:t1, :],
                    )
                    tbl = nc.dram_tensor(f"table{layer}{hseg}", [nrows, HID],
                                         tdt, addr_space="Shared")
                    agi = nc.gpsimd.collective_compute(
                        "AllGather",
                        mybir.AluOpType.bypass,
                        replica_groups=[list(range(N_CORES))],
                        ins=[agin[:].opt()],
                        outs=[tbl[:].opt()],
                    )
                    agb_name[0] = agi.ins.name
                    tables.append(
                        (tbl, shadow_of(tbl, f"tblsh{layer}{hseg}",
                                        [nrows, HID], tdt, "Shared")))

                # tokens last: a token DMA waits its AllGather, and the SP
                # stream is in-order — emitting it mid-transform would stall
                # the half-B xloads behind AG_A.
                for hseg, (tbl, _sh) in enumerate(tables):
                    tok = tokpool.tile([P, 2], tdt, tag=f"tok{hseg}")
                    nc.sync.dma_start(
                        out=tok[:, :1],
                        in_=tbl[:].rearrange("(t p) h -> p t h", p=P)[:, :1, :1],
                    )
                    tokens.append(tok)

                # ---- gather + reduce, two passes by source half ----
                # The first PREPN calls of pass A are PREPARE_ONLY so their
                # desc-gen overlaps the transform + AllGather above.
                pdrams = []
                pstate = dict(left=PREPN, queues=set(), pending=False,
                              bq=[0] * NQ)

                def flush_triggers(tok_ap):
                    if pstate["pending"]:
                        for tq in sorted(pstate["queues"]):
                            ti = nc.gpsimd.trigger_dma(
                                count=None, queue_num=tq,
                                signals_writable=[tok_ap[:]],
                            )
                            last_trig[tq] = ti.ins.name
                        pstate["queues"].clear()
                        pstate["pending"] = False
                        pstate["bq"] = [0] * NQ

                def emit_call(g, s0, kc, isb, o0, tview, shview, use_prep,
                              tok_ap, waits):
                    if use_prep:
                        allowed = [j for j in range(NQ)
                                   if pstate["bq"][j] < NSEM_PQ]
                        q = pick_q(kc, allowed)
                        pstate["bq"][q] += 1
                        sem, tgt = next_sem(q)
                        pi = nc.gpsimd.dma_gather(
                            out_ap=g[:, s0 : s0 + kc, :], in_ap=shview,
                            idxs_ap=isb[:, o0 * 8 : (o0 + kc) * 8],
                            num_idxs=kc * P, num_idxs_reg=kc * P,
                            elem_size=HID, queue_num=q,
                            prepare_only=True, sem=sem,
                        )
                        order_after(pi, last_gen0[q], last_trig[q])
                        waits.append((sem, tgt))
                        pstate["queues"].add(q)
                        pstate["left"] -= 1
                        pstate["pending"] = True
                    else:
                        q = pick_q(kc)
                        flush_triggers(tok_ap)
                        gi = nc.gpsimd.dma_gather(
                            out_ap=g[:, s0 : s0 + kc, :], in_ap=tview,
                            idxs_ap=isb[:, o0 * 8 : (o0 + kc) * 8],
                            num_idxs=kc * P, num_idxs_reg=kc * P,
                            elem_size=HID, queue_num=q,
                        )
                        last_gen0[q] = order_after(gi, last_trig[q])

                def emit_group(part, t, K, g0, first, isb, off, tview, shview,
                               use_prep, tok_ap, anchor):
                    cap = CHUNK if use_prep else SCAP
                    Kg = min(cap, K - g0)
                    if use_prep:
                        gt = ppool.tile([P, CHUNK, HID], tdt, tag="p")
                    else:
                        gt = gpool.tile([P, SCAP, HID], tdt, tag="g")
                    g = gt[:]
                    waits = []
                    s0 = 0
                    while s0 < Kg:
                        kc = min(CHUNK, Kg - s0)
                        emit_call(g, s0, kc, isb, off + g0 + s0, tview, shview,
                                  use_prep, tok_ap, waits)
                        s0 += kc
                    eng = nc.vector if waits else nc.any
                    wait_names = []
                    for sem, tgt in waits:
                        wi = nc.vector.wait_ge(sem, tgt)
                        # anchor: the scheduler must not hoist this engine
                        # wait above the work feeding the trigger's deps
                        wait_names.append(order_after(wi, anchor))
                    ofn = (lambda i: order_after(i, *wait_names)) \
                        if wait_names else None
                    if first:
                        _tree_reduce_into(nc, g, Kg, part[:, t, :], eng=eng,
                                          order_fn=ofn)
                    else:
                        tmp = tpool.tile([P, P], tdt, tag="gtmp")
                        _tree_reduce_into(nc, g, Kg, tmp[:, :HID], eng=eng,
                                          order_fn=ofn)
                        ai = eng.tensor_add(part[:, t, :], part[:, t, :],
                                            tmp[:, :HID])
                        if ofn:
                            ofn(ai)
                    return Kg

                def emit_pass(half, Ks, isb, anchor):
                    part = partpool.tile([P, MT, HID], tdt, tag="part")
                    tview = tables[half][0][:]
                    shview = tables[half][1][:]
                    tok_ap = tokens[half]
                    offs = [0] * MT
                    for t in range(1, MT):
                        offs[t] = offs[t - 1] + Ks[t - 1]
                    for t in range(MT):
                        K = Ks[t]
                        if K == 0:
                            nc.vector.memset(part[:, t, :], 0.0)
                            continue
                        off = offs[t]
                        g0 = 0
                        first = True
                        while g0 < K:
                            use_prep = half == 0 and pstate["left"] > 0
                            g0 += emit_group(part, t, K, g0, first, isb, off,
                                             tview, shview, use_prep, tok_ap,
                                             anchor)
                            first = False
                    flush_triggers(tok_ap)
                    pd = nc.dram_tensor(f"pd{layer}{half}", [BLK, HID], tdt)
                    pdw = nc.sync.dma_start(
                        out=pd[:].rearrange("(t p) h -> p t h", p=P),
                        in_=part[:],
                    )
                    pdsh = shadow_of(pd, f"pdsh{layer}{half}", [BLK, HID],
                                     tdt, "Local")
                    pdrams.append((pd, pdsh, pdw.ins.name))

                ag_anchor = agb_name[0]
                emit_pass(0, KA, idxasb, ag_anchor)
                emit_pass(1, KB, idxbsb, ag_anchor)

                # ---- canonicalize (prepped) + merge + bias/relu ----
                accA = accpool.tile([P, MT, HID], tdt, tag="accA")
                accB = accpool.tile([P, MT, HID], tdt, tag="accB")
                canon_waits = {0: [], 1: []}
                canon_queues = set()
                canon_bq = [0] * NQ
                pd_anchors = (pdrams[0][2], pdrams[1][2])
                for half, (dst, pdt, mapsb) in enumerate(
                    ((accA, pdrams[0], mapasb), (accB, pdrams[1], mapbsb))
                ):
                    for c0 in range(0, MT, CHUNK):
                        cc = min(CHUNK, MT - c0)
                        allowed = [j for j in range(NQ)
                                   if canon_bq[j] < NSEM_PQ]
                        q = pick_q(cc, allowed)
                        canon_bq[q] += 1
                        sem, tgt = next_sem(q)
                        pi = nc.gpsimd.dma_gather(
                            out_ap=dst[:, c0 : c0 + cc, :], in_ap=pdt[1][:],
                            idxs_ap=mapsb[:, c0 * 8 : (c0 + cc) * 8],
                            num_idxs=cc * P, num_idxs_reg=cc * P,
                            elem_size=HID, queue_num=q,
                            prepare_only=True, sem=sem,
                        )
                        order_after(pi, last_gen0[q], last_trig[q])
                        canon_queues.add(q)
                        canon_waits[half].append((c0, c0 + cc, sem, tgt))
                ptokA = tokpool.tile([P, 2], tdt, tag="ptokA")
                nc.sync.dma_start(
                    out=ptokA[:, :1],
                    in_=pdrams[0][0][:].rearrange("(t p) h -> p t h", p=P)[
                        :, :1, :1],
                )
                ptokB = tokpool.tile([P, 2], tdt, tag="ptokB")
                nc.sync.dma_start(
                    out=ptokB[:, :1],
                    in_=pdrams[1][0][:].rearrange("(t p) h -> p t h", p=P)[
                        :, :1, :1],
                )
                for tq in sorted(canon_queues):
                    ti = nc.gpsimd.trigger_dma(
                        count=None, queue_num=tq,
                        signals_writable=[ptokA[:], ptokB[:]],
                    )
                    last_trig[tq] = ti.ins.name

                hnew = hnewpool.tile([P, MT, HID], dt.float32, tag="hnew")
                bsb = b1sb if layer == 1 else b2sb
                # quarter-width post ops so downstream per-tile consumers
                # (transposes / next matmul / fc) can start early
                QCH = max(1, (MT + 3) // 4)
                done_sems = {}
                for c0 in range(0, MT, QCH):
                    c1 = min(MT, c0 + QCH)
                    sl = slice(c0, c1)
                    w = c1 - c0
                    cw_names = []
                    for half in (0, 1):
                        for (a0, a1, sem, tgt) in canon_waits[half]:
                            if a0 < c1 and a1 > c0:
                                if (sem, tgt) in done_sems:
                                    cw_names.append(done_sems[(sem, tgt)])
                                else:
                                    wi = nc.vector.wait_ge(sem, tgt)
                                    nm = order_after(wi, *pd_anchors)
                                    done_sems[(sem, tgt)] = nm
                                    cw_names.append(nm)
                    mi = nc.vector.tensor_add(hnew[:, sl, :], accA[:, sl, :],
                                              accB[:, sl, :])
                    order_after(mi, *cw_names)
                    nc.vector.tensor_add(hnew[:, sl, :], hnew[:, sl, :],
                                         hs[:, sl, :])  # self-loop term
                    dv3 = dvsb[:, sl].to_broadcast([P, w, HID])
                    nc.vector.tensor_tensor(hnew[:, sl, :], hnew[:, sl, :],
                                            dv3, op=mybir.AluOpType.mult)
                    b3 = bsb[:].rearrange("p (o h) -> p o h", o=1).to_broadcast(
                        [P, w, HID])
                    nc.vector.tensor_tensor(hnew[:, sl, :], hnew[:, sl, :],
                                            b3, op=mybir.AluOpType.add)
                    nc.scalar.activation(hnew[:, sl, :], hnew[:, sl, :],
                                         mybir.ActivationFunctionType.Relu)
                h_prev = hnew

            # ---- classifier ----
            outsb = outpool.tile([P, MT, CLS], dt.float32, tag="outsb")
            for m in range(MT):
                pst = pstpool.tile([P, P], dt.float32, tag="tr")
                nc.tensor.transpose(pst[:], h_prev[:, m, :], ident[:])
                hT = tpool.tile([P, P], dt.bfloat16, tag="hT")
                nc.any.tensor_copy(hT[:], pst[:])
                ps2 = pspool.tile([P, CLS], dt.float32, tag="mm2")
                nc.tensor.matmul(ps2[:], hT[:], wfcsb[:], start=True, stop=True)
                nc.vector.tensor_add(outsb[:, m, :], ps2[:], bfcsb[:])
                if m % 12 == 11 or m == MT - 1:
                    m0 = (m // 12) * 12
                    nc.sync.dma_start(
                        out=out[:].rearrange("(t p) c -> p t c", p=P)[
                            :, m0 : m + 1, :
                        ],
                        in_=outsb[:, m0 : m + 1, :],
                    )

    # The tile pass emits the prepare_only DMASW pre-bumps (InstIncSwdgeSem)
    # with queue_num=0 regardless of the prep's queue; the ucode's per-queue
    # ring bookkeeping (and the sim's queue locks) need the real queue.
    import concourse.bass_isa as _bisa
    lanes_per_q = max(1, 8 // NQ)
    for _fn in nc.m.functions:
        for _blk in _fn.blocks:
            for _inst in _blk.instructions:
                if isinstance(_inst, _bisa.InstIncSwdgeSem) and \
                        _inst._mode == "add":
                    for _n, _v in zip(_inst._sem_names, _inst._sem_values):
                        if _v and _n.startswith("DMASW"):
                            lane = int(_n[5:].split("_")[0])
                            _inst.queue_num = lane // lanes_per_q
                            break

    nc.compile()
    return nc


# ----------------------------------------------------------------------------
# Entry point
# ----------------------------------------------------------------------------

_CACHE = {}


def _get_graph(meta):
    key = (meta["IN"], meta["HID"], meta["CLS"], meta["BLK"], meta["NPAD"],
           meta["KA"], meta["KB"])
    if key not in _CACHE:
        _CACHE[key] = _build(meta)
    return _CACHE[key]


def kernel(x, edge_index, W1, b1, W2, b2, Wfc, bfc, _want_profile=False,
           _stage="full"):
    x = np.asarray(x, dtype=np.float32)
    in_maps, meta = _preprocess(np.asarray(x), np.asarray(edge_index),
                                np.asarray(W1), np.asarray(b1),
                                np.asarray(W2), np.asarray(b2),
                                np.asarray(Wfc), np.asarray(bfc))
    nc = _get_graph(meta)
    res = run_bass_kernel_spmd(nc, in_maps, core_ids=list(range(N_CORES)),
                               trace=_want_profile)
    N, CLS = meta["N"], meta["CLS"]
    BLK_RAW = meta["BLK_RAW"]
    full = np.empty((N, CLS), dtype=np.float32)
    for r in range(N_CORES):
        lo = r * BLK_RAW
        hi = min(N, (r + 1) * BLK_RAW)
        if hi > lo:
            full[lo:hi] = res.results[r]["out"][1 : 1 + hi - lo]
    if _want_profile:
        return full, res
    return full
